# revision 1
# baseline (speedup 1.0000x reference)
"""CoverageAttention fused Trainium2 kernel v2 (8 NeuronCores, data-parallel).

Differences vs v1 baseline:
  - ONE packed output tensor out_o[bpc, 3*S] = [ctx | attn | covn] per batch
    (v1's three separate ExternalOutputs triggered a ~130ms/call slow path in
    the axon PJRT dispatch; measured: any 3-output kernel pays it, 1-output
    kernels run at the harness floor).
  - Inputs packed into 4 tensors (enct, encn, auxr, auxf): fewer args.
  - All big DMA loads are contiguous per partition (host pre-tiles encT into
    [bpc, NSB, P, KT*SB] and the weights into partition-major layouts).
  - Context encoder tiles loaded as [P, E] 1MB contiguous blocks (16/batch
    instead of 64 strided 256KB tiles).
  - Mask bias added to scores with one DVE add per batch instead of a
    K=1/M=1 matmul per s-block.
  - attn transpose matmuls run in plain fp32 (K=1, N=2 — free); f32r
    rounding happens at the psum->SBUF copy of attnT.

Compute structure (per batch, per core; identical math to v1):
  energyT[h,s] = tanh(W_h^T enc^T + dec_proj bias + cov outer-product)
  scores = v^T energyT (PE), + mask bias (DVE)
  softmax row [1,S] (DVE/ACT), coverage_new = cov + attn
  attnT via 16 K=1 PE transposes; ctx[1,E] = sum_st attnT_st^T @ encN tiles
Batches software-pipelined: tail(b-1) emitted after main(b).
"""

import numpy as np

P = 128
B_FULL = 64
S_FULL = 2048
H = 1024
E = 2048
N_CORES = 8
SB = 512

KT = E // P          # 16 contraction tiles (main GEMM)
HT = H // P          # 8 h-tiles
DKT = H // P         # 8 contraction tiles (dec_proj)
NSB = S_FULL // SB   # 4 s-blocks
NST = S_FULL // P    # 16 s-tiles

# aux_r layout: [P, W] float32r, all contiguous per partition
OFF_WH = 0                       # [P, KT*H]   wh_tiled
OFF_WD = OFF_WH + KT * H         # [P, DKT*H]  wd_tiled
OFF_VV = OFF_WD + DKT * H        # [P, HT]     v tiled
OFF_DECT = OFF_VV + HT           # [P, DKT*BPC2] decT tiled
BPC2 = 8
OFF_ROWS = OFF_DECT + DKT * BPC2  # [P, S] row block:
#   rows 0..bpc-1 : covr (f32r coverage)
#   row  8        : wc (cols 0:H)
#   row  9        : one_r (cols 0:2) = [1, 0]
W_AUX = OFF_ROWS + S_FULL

_CACHE: dict = {}


def _round_f32r(a: np.ndarray) -> np.ndarray:
    """Round fp32 to float32r (11 explicit mantissa bits), round-to-nearest-even."""
    u = np.ascontiguousarray(a, dtype=np.float32).view(np.uint32)
    lsb = (u >> 12) & np.uint32(1)
    r = (u + np.uint32(0x7FF) + lsb) & np.uint32(0xFFFFF000)
    return r.view(np.float32)


def _build_nc(bpc: int, S: int):
    import concourse.mybir as mybir
    import concourse.tile as tile
    from concourse import bacc

    f32 = mybir.dt.float32
    f32r = mybir.dt.float32r
    Tanh = mybir.ActivationFunctionType.Tanh
    Exp = mybir.ActivationFunctionType.Exp
    X = mybir.AxisListType.X

    nsb = S // SB
    nst = S // P

    nc = bacc.Bacc(None, target_bir_lowering=False)

    enct = nc.dram_tensor("enct", [bpc, nsb, P, KT * SB], f32r,
                          kind="ExternalInput")
    encn = nc.dram_tensor("encn", [bpc, S, E], f32r, kind="ExternalInput")
    auxr = nc.dram_tensor("auxr", [P, W_AUX], f32r, kind="ExternalInput")
    auxf = nc.dram_tensor("auxf", [2 * bpc + 1, S], f32, kind="ExternalInput")
    out_o = nc.dram_tensor("out_o", [bpc, 3 * S], f32, kind="ExternalOutput")

    with tile.TileContext(nc) as tc:
        with (
            tc.tile_pool(name="big", bufs=1) as big,
            tc.tile_pool(name="enctp", bufs=2) as enctp,
            tc.tile_pool(name="epool", bufs=2) as epool,
            tc.tile_pool(name="cpool", bufs=2) as cpool,
            tc.tile_pool(name="rows2", bufs=2) as rows2,
            tc.tile_pool(name="rowsml", bufs=1) as rowsml,
            tc.tile_pool(name="singles", bufs=1) as singles,
            tc.tile_pool(name="mainps", bufs=3, space="PSUM") as mainps,
            tc.tile_pool(name="scoresps", bufs=1, space="PSUM") as scoresps,
            tc.tile_pool(name="smallps", bufs=1, space="PSUM") as smallps,
            tc.tile_pool(name="ctxps", bufs=2, space="PSUM") as ctxps,
        ):
            # ---------------- prologue: weights (all contiguous loads) ------
            wh_sb = big.tile([P, KT * H], f32r, tag="wh")
            nc.sync.dma_start(wh_sb, auxr[:, OFF_WH:OFF_WH + KT * H])
            v_sb = singles.tile([P, HT], f32r)
            nc.sync.dma_start(v_sb, auxr[:, OFF_VV:OFF_VV + HT])
            dect_sb = singles.tile([P, DKT * BPC2], f32r)
            nc.sync.dma_start(dect_sb, auxr[:, OFF_DECT:OFF_DECT + DKT * BPC2])
            wc_sb = singles.tile([1, H], f32r)
            nc.sync.dma_start(wc_sb, auxr[8:9, OFF_ROWS:OFF_ROWS + H])
            one_f = singles.tile([1, 2], f32)
            nc.sync.dma_start(one_f, auxf[2 * bpc:2 * bpc + 1, 0:2])

            wdt = enctp.tile([P, DKT * H], f32r, tag="enct")
            nc.sync.dma_start(wdt, auxr[:, OFF_WD:OFF_WD + DKT * H])
            dp_sb = singles.tile([P, HT, BPC2], f32)
            for ht in range(HT):
                dps = smallps.tile([P, BPC2], f32, tag="smallps")
                for k in range(DKT):
                    nc.tensor.matmul(
                        dps,
                        wdt[:, k * H + ht * P:k * H + (ht + 1) * P],
                        dect_sb[:, k * BPC2:(k + 1) * BPC2],
                        start=(k == 0),
                        stop=(k == DKT - 1),
                    )
                nc.vector.tensor_copy(dp_sb[:, ht, :], dps)

            # ---------------- per-batch passes ----------------
            state = {}

            def emit_main(b):
                covr_row = rowsml.tile([1, S], f32r, tag="covr")
                nc.sync.dma_start(covr_row,
                                  auxr[b:b + 1, OFF_ROWS:OFF_ROWS + S])
                covf_row = rowsml.tile([1, S], f32, tag="covf")
                nc.sync.dma_start(covf_row, auxf[b:b + 1, :])
                maskb_row = rowsml.tile([1, S], f32, tag="maskb")
                nc.sync.dma_start(maskb_row, auxf[bpc + b:bpc + b + 1, :])
                scores_row = rows2.tile([1, S], f32, tag="scores")
                for sb in range(nsb):
                    et = enctp.tile([P, KT * SB], f32r, tag="enct")
                    nc.sync.dma_start(et, enct[b, sb, :, :])
                    sc_ps = scoresps.tile([1, SB], f32, tag="scoresps")
                    for ht in range(HT):
                        mp = mainps.tile([P, SB], f32, tag="mainps")
                        for k in range(KT):
                            nc.tensor.matmul(
                                mp,
                                wh_sb[:, k * H + ht * P:k * H + (ht + 1) * P],
                                et[:, k * SB:(k + 1) * SB],
                                start=(k == 0),
                                stop=False,
                            )
                        # += coverage[s] * W_c[h]  (outer product, K=1)
                        nc.tensor.matmul(
                            mp,
                            wc_sb[0:1, ht * P:(ht + 1) * P],
                            covr_row[0:1, sb * SB:(sb + 1) * SB],
                            start=False,
                            stop=True,
                        )
                        en = epool.tile([P, SB], f32r, tag="energy")
                        nc.scalar.activation(
                            en, mp, Tanh, bias=dp_sb[:, ht, b:b + 1]
                        )
                        nc.tensor.matmul(
                            sc_ps,
                            v_sb[:, ht:ht + 1],
                            en,
                            start=(ht == 0),
                            stop=(ht == HT - 1),
                        )
                    nc.scalar.copy(scores_row[0:1, sb * SB:(sb + 1) * SB], sc_ps)
                # += (mask-1)*1e4  (one DVE add for the whole row)
                nc.vector.tensor_add(scores_row, scores_row, maskb_row)
                state[b] = (scores_row, covf_row)

            def emit_tail(b):
                scores_row, covf_row = state.pop(b)
                nmax = singles.tile([1, 1], f32, tag=f"nmax{b % 2}")
                nc.vector.reduce_max(nmax, scores_row[0:1, :], axis=X,
                                     negate=True)
                attn_u = rowsml.tile([1, S], f32, tag="attnu")
                den = singles.tile([1, 1], f32, tag=f"den{b % 2}")
                nc.scalar.activation(
                    attn_u, scores_row[0:1, :], Exp, bias=nmax[0:1, 0:1],
                    accum_out=den[0:1, 0:1],
                )
                rden = singles.tile([1, 1], f32, tag=f"rden{b % 2}")
                nc.vector.reciprocal(rden, den)
                # attn = exp(...) / den, in place; this is the attn output
                nc.vector.tensor_scalar_mul(attn_u, attn_u, rden[0:1, 0:1])
                nc.sync.dma_start(out_o[b:b + 1, S:2 * S], attn_u)
                # coverage_new = coverage + attn (in place on covf tile)
                nc.vector.tensor_add(covf_row, covf_row, attn_u)
                nc.sync.dma_start(out_o[b:b + 1, 2 * S:3 * S], covf_row)
                # attn -> partitions (16 trivial K=1 transposing matmuls, fp32;
                # f32r rounding happens at the at_sb copy below)
                atp = smallps.tile([P, 2 * nst], f32, tag="smallps")
                for st in range(nst):
                    nc.tensor.matmul(
                        atp[:, 2 * st:2 * st + 2],
                        attn_u[0:1, st * P:(st + 1) * P],
                        one_f[0:1, :],
                        start=True,
                        stop=True,
                    )
                at_sb = epool.tile([P, nst], f32r, tag="attnT")
                nc.vector.tensor_copy(
                    at_sb,
                    atp[:, :].rearrange("p (t two) -> p t two", two=2)[:, :, 0],
                )
                # context: 4 psum banks accumulate over 16 s-tiles
                ctx_row = rowsml.tile([1, E], f32, tag="ctxrow")
                for half in range(2):
                    cps = [
                        ctxps.tile([1, SB], f32, tag="ctxps", name=f"cps{j}")
                        for j in range(2)
                    ]
                    for st in range(nst):
                        ce = cpool.tile([P, E // 2], f32r, tag="ctxenc")
                        nc.sync.dma_start(
                            ce,
                            encn[b, st * P:(st + 1) * P,
                                 half * (E // 2):(half + 1) * (E // 2)])
                        for j in range(2):
                            nc.tensor.matmul(
                                cps[j],
                                at_sb[:, st:st + 1],
                                ce[:, j * SB:(j + 1) * SB],
                                start=(st == 0),
                                stop=(st == nst - 1),
                            )
                    for j in range(2):
                        nc.scalar.copy(
                            ctx_row[0:1, half * E // 2 + j * SB:
                                    half * E // 2 + (j + 1) * SB],
                            cps[j])
                nc.sync.dma_start(out_o[b:b + 1, 0:E], ctx_row)

            # software pipeline: tail(b-1) is emitted after main(b)
            for b in range(bpc + 1):
                if b < bpc:
                    emit_main(b)
                if b >= 1:
                    emit_tail(b - 1)

    nc.compile()
    return nc


def _get_nc(bpc: int, S: int):
    key = (bpc, S)
    if key not in _CACHE:
        _CACHE[key] = _build_nc(bpc, S)
    return _CACHE[key]


def _prepare_in_maps(decoder_hidden, encoder_outputs, coverage, mask,
                     W_h, W_d, W_c, v, n_cores: int):
    """Host-side prep: shard over batch, pre-tile everything contiguous."""
    dec = np.asarray(decoder_hidden, dtype=np.float32)
    cov = np.asarray(coverage, dtype=np.float32)
    msk = np.asarray(mask)
    B = dec.shape[0]
    bpc = B // n_cores
    S = cov.shape[1]
    nsb = S // SB

    # weight tiles (partition-major, contiguous)
    wh_t = _round_f32r(np.asarray(W_h, np.float32)
                       .reshape(KT, P, H).transpose(1, 0, 2)
                       .reshape(P, KT * H))
    wd_t = _round_f32r(np.asarray(W_d, np.float32)
                       .reshape(DKT, P, H).transpose(1, 0, 2)
                       .reshape(P, DKT * H))
    v_t = _round_f32r(np.asarray(v, np.float32)[:, 0]
                      .reshape(HT, P).T.copy())          # [P, HT]
    wc_r = _round_f32r(np.asarray(W_c, np.float32))      # [1, H]
    maskb = ((msk.astype(np.float32) - 1.0) * 10000.0)   # [B, S] f32

    enc = np.asarray(encoder_outputs, dtype=np.float32)
    in_maps = []
    for c in range(n_cores):
        sl = slice(c * bpc, (c + 1) * bpc)
        enc_r = _round_f32r(enc[sl])                     # [bpc, S, E]
        # [bpc, nsb, P, KT*SB]: enct[b,sb,p,k*SB+s] = enc[b, sb*SB+s, k*P+p]
        enct = np.ascontiguousarray(
            enc_r.reshape(bpc, nsb, SB, KT, P).transpose(0, 1, 4, 3, 2)
        ).reshape(bpc, nsb, P, KT * SB)

        dslice = dec[sl].T                               # [H, bpc]
        if dslice.shape[1] < BPC2:
            dslice = np.concatenate(
                [dslice, np.zeros((H, BPC2 - dslice.shape[1]), np.float32)],
                axis=1)
        dect_t = _round_f32r(
            dslice.reshape(DKT, P, BPC2).transpose(1, 0, 2)
            .reshape(P, DKT * BPC2))

        auxr = np.zeros((P, W_AUX), np.float32)
        auxr[:, OFF_WH:OFF_WH + KT * H] = wh_t
        auxr[:, OFF_WD:OFF_WD + DKT * H] = wd_t
        auxr[:, OFF_VV:OFF_VV + HT] = v_t
        auxr[:, OFF_DECT:OFF_DECT + DKT * BPC2] = dect_t
        auxr[0:bpc, OFF_ROWS:OFF_ROWS + S] = _round_f32r(cov[sl])
        auxr[8, OFF_ROWS:OFF_ROWS + H] = wc_r[0]
        auxr[9, OFF_ROWS] = 1.0

        one_row = np.zeros((1, S), np.float32)
        one_row[0, 0] = 1.0
        auxf = np.concatenate([cov[sl], maskb[sl], one_row],
                              axis=0)  # [2*bpc+1, S] f32

        in_maps.append({
            "enct": enct,
            "encn": enc_r,
            "auxr": auxr,
            "auxf": auxf,
        })
    return in_maps, bpc


def kernel(decoder_hidden, encoder_outputs, coverage, mask, W_h, W_d, W_c, v):
    from concourse.bass_utils import run_bass_kernel_spmd

    in_maps, bpc = _prepare_in_maps(
        decoder_hidden, encoder_outputs, coverage, mask, W_h, W_d, W_c, v,
        N_CORES,
    )
    S = np.asarray(coverage).shape[1]
    nc = _get_nc(bpc, S)
    res = run_bass_kernel_spmd(nc, in_maps, core_ids=list(range(N_CORES)))
    out = np.concatenate([r["out_o"] for r in res.results], axis=0)
    context = out[:, 0:E]
    attn = out[:, S:S + S]
    covn = out[:, 2 * S:3 * S]
    return context, attn, covn

